# revision 8
# baseline (speedup 1.0000x reference)
"""Trainium2 Bass kernel for the DeLight-style GPT language model.

Model: 8-layer dense transformer, per-layer head counts [6,7,8,9,9,10,11,12]
and FFN widths [512,768,1280,1536,2048,2304,2816,3072], E=768, T=1024, B=4,
V=50257, returning (logits [B,T,V] f32, mean CE loss).

Sharding (8 cores, no collectives): core c handles batch element b = c % 4
and vocab half vh = c // 4. The transformer forward for batch element b is
computed redundantly on the core pair (b, b+4) — this avoids any cross-core
communication — and the dominant lm_head GEMM + softmax-sum statistics are
split 8 ways across (batch, vocab-half). The host assembles logits from the
8 vocab-half shards and combines per-half exp-sums into the loss (O(B*T)
work only).

All GEMMs run in bf16 with fp32 PSUM accumulation; the residual stream,
layernorm statistics and softmax denominators stay fp32.
"""
import os
from contextlib import ExitStack

import numpy as np
import ml_dtypes

import concourse.bass as bass
import concourse.tile as tile
from concourse import bacc, mybir
from concourse.bass_utils import run_bass_kernel_spmd
from concourse.masks import make_identity

F32 = mybir.dt.float32
BF16 = mybir.dt.bfloat16
I32 = mybir.dt.int32
AF = mybir.ActivationFunctionType
AX = mybir.AxisListType
OP = mybir.AluOpType

V, E, HS, T, B, NL = 50257, 768, 64, 1024, 4, 8
HEADS = [6, 7, 8, 9, 9, 10, 11, 12]
FFN = [512, 768, 1280, 1536, 2048, 2304, 2816, 3072]
LN_EPS = 1e-5
KT = E // 128            # 6 contraction k-tiles over E
NTB = T // 128           # 8 token blocks
VHALF0 = 25129           # vocab half sizes (half1 = V - VHALF0)
VH = 25600               # padded per-core vocab half (50 x 512)
NVC = VH // 512          # vocab chunks
FC = 512                 # FFN intermediate chunk


def _bcast(ap2d, parts=128):
    """Partition-broadcast a [1, ...] DRAM/SBUF AP to `parts` partitions."""
    return bass.AP(tensor=ap2d.tensor, offset=ap2d.offset,
                   ap=[[0, parts]] + [list(p) for p in ap2d.ap])


def build_nc():
    nc = bacc.Bacc("TRN2", target_bir_lowering=False, debug=False)

    # ---- I/O -----------------------------------------------------------
    idx32 = nc.declare_dram_parameter("idx32", [128, NTB], I32, isOutput=False)
    tok = nc.declare_dram_parameter("tok", [V, E], BF16, isOutput=False)
    pos = nc.declare_dram_parameter("pos", [T, E], BF16, isOutput=False)
    cmask = nc.declare_dram_parameter("cmask", [128, 128], BF16, isOutput=False)
    wqkv, projw, w1, w2, b1, lngb = [], [], [], [], [], []
    for l in range(NL):
        h, f = HEADS[l], FFN[l]
        D = h * HS
        nkp = (D + 127) // 128
        wqkv.append(nc.declare_dram_parameter(f"wqkv{l}", [E, 3 * D], BF16, isOutput=False))
        projw.append(nc.declare_dram_parameter(f"projw{l}", [nkp * 128, E], BF16, isOutput=False))
        w1.append(nc.declare_dram_parameter(f"w1_{l}", [E, f], BF16, isOutput=False))
        w2.append(nc.declare_dram_parameter(f"w2_{l}", [f, E], BF16, isOutput=False))
        b1.append(nc.declare_dram_parameter(f"b1_{l}", [128, f // 128], F32, isOutput=False))
        # rows: ln1_g, ln1_b, ln2_g, ln2_b, proj_b, b2
        lngb.append(nc.declare_dram_parameter(f"lngb{l}", [6, E], BF16, isOutput=False))
    lnfgb = nc.declare_dram_parameter("lnfgb", [2, E], BF16, isOutput=False)
    lmw = nc.declare_dram_parameter("lmw", [E, VH], BF16, isOutput=False)
    lmb = nc.declare_dram_parameter("lmb", [1, VH], F32, isOutput=False)

    logits = nc.declare_dram_parameter("logits", [T, VH], F32, isOutput=True)
    sumexp = nc.declare_dram_parameter("sumexp", [NTB, 128], F32, isOutput=True)

    with tile.TileContext(nc) as tc, ExitStack() as top:
        persist = top.enter_context(tc.tile_pool(name="persist", bufs=1))
        hT_pool = top.enter_context(tc.tile_pool(name="hTp", bufs=1))

        x = persist.tile([128, NTB, E], F32, tag="x")
        idx_sb = persist.tile([128, NTB], I32, tag="idx")
        ident = persist.tile([128, 128], BF16, tag="ident")
        make_identity(nc, ident[:])
        eps_sb = persist.tile([128, 1], F32, tag="eps")
        nc.vector.memset(eps_sb[:], LN_EPS)
        ones64 = persist.tile([1, 64], F32, tag="ones")
        nc.vector.memset(ones64[:], 1.0)
        cm_sb = persist.tile([128, 128], BF16, tag="cmask")
        nc.sync.dma_start(cm_sb[:], cmask[:, :])
        sums_final = persist.tile([128, NTB], F32, tag="sumsf")

        nc.sync.dma_start(idx_sb[:], idx32[:, :])

        def layer_norm(gb_sb, grow, brow, scratch, psum_pool, hT):
            """LN over x -> hT [128, KT, T] bf16 (transposed)."""
            for tb in range(NTB):
                stats = scratch.tile([128, 3, 6], F32, tag="stats")
                for i in range(3):
                    nc.vector.bn_stats(out=stats[:, i, :], in_=x[:, tb, i * 256:(i + 1) * 256])
                mv = scratch.tile([128, 2], F32, tag="mv")
                nc.vector.bn_aggr(out=mv[:], in_=stats[:])
                r = scratch.tile([128, 1], F32, tag="r")
                nc.scalar.activation(out=r[:], in_=mv[:, 1:2], func=AF.Sqrt, bias=eps_sb[:])
                nc.vector.reciprocal(r[:], r[:])
                hn = scratch.tile([128, E], F32, tag="hn")
                nc.vector.tensor_scalar(out=hn[:], in0=x[:, tb, :], scalar1=mv[:, 0:1],
                                        scalar2=r[:, 0:1], op0=OP.subtract, op1=OP.mult)
                nc.vector.tensor_tensor(out=hn[:], in0=hn[:], in1=gb_sb[:, grow, :], op=OP.mult)
                hnb = scratch.tile([128, E], BF16, tag="hnb")
                nc.vector.tensor_tensor(out=hnb[:], in0=hn[:], in1=gb_sb[:, brow, :], op=OP.add)
                for k in range(KT):
                    pt = psum_pool.tile([128, 128], BF16, tag="pstr", name="ptt")
                    nc.tensor.transpose(pt[:], hnb[:, k * 128:(k + 1) * 128], ident[:])
                    nc.vector.tensor_copy(out=hT[:, k, tb * 128:(tb + 1) * 128], in_=pt[:])

        # ================= phase 1: embedding + transformer ==============
        with ExitStack() as p1:
            wq_pool = p1.enter_context(tc.tile_pool(name="wq", bufs=1))
            w1_pool = p1.enter_context(tc.tile_pool(name="w1", bufs=2))
            w2_pool = p1.enter_context(tc.tile_pool(name="w2", bufs=2))
            pw_pool = p1.enter_context(tc.tile_pool(name="pw", bufs=1))
            gb_pool = p1.enter_context(tc.tile_pool(name="gb", bufs=1))
            b1_pool = p1.enter_context(tc.tile_pool(name="b1", bufs=2))
            qk_pool = p1.enter_context(tc.tile_pool(name="qk", bufs=1))
            v_pool = p1.enter_context(tc.tile_pool(name="v", bufs=1))
            v65_pool = p1.enter_context(tc.tile_pool(name="v65", bufs=2))
            ex_pool = p1.enter_context(tc.tile_pool(name="ex", bufs=2))
            oT_pool = p1.enter_context(tc.tile_pool(name="oT", bufs=1))
            aT_pool = p1.enter_context(tc.tile_pool(name="aT", bufs=2))
            sc_pool = p1.enter_context(tc.tile_pool(name="scr", bufs=2))
            rc_pool = p1.enter_context(tc.tile_pool(name="rc", bufs=2))

            # ---- embedding ----
            with tc.tile_pool(name="pse", bufs=2, space="PSUM") as pse:
                for tb in range(NTB):
                    emb = sc_pool.tile([128, E], BF16, tag="emb")
                    nc.gpsimd.indirect_dma_start(
                        out=emb[:], out_offset=None, in_=tok[:, :],
                        in_offset=bass.IndirectOffsetOnAxis(ap=idx_sb[:, tb:tb + 1], axis=0))
                    pt = sc_pool.tile([128, E], BF16, tag="post")
                    nc.sync.dma_start(pt[:], pos[tb * 128:(tb + 1) * 128, :])
                    nc.vector.tensor_tensor(out=x[:, tb, :], in0=emb[:], in1=pt[:], op=OP.add)

            for l in range(NL):
                h, f = HEADS[l], FFN[l]
                D = h * HS
                ndt = (D + 127) // 128

                gb_sb = gb_pool.tile([128, 6, E], BF16, tag="gb")
                nc.sync.dma_start(gb_sb[:], _bcast(lngb[l][:, :]))
                b1_sb = b1_pool.tile([128, 24], F32, tag="b1", name="b1t")[:, :f // 128]
                nc.sync.dma_start(b1_sb, b1[l][:, :])

                hT = hT_pool.tile([128, KT, T], BF16, tag="hT")
                with tc.tile_pool(name=f"psln1_{l}", bufs=2, space="PSUM") as pp:
                    layer_norm(gb_sb, 0, 1, sc_pool, pp, hT)

                # ---- qkv ----
                wq_sb = wq_pool.tile([128, KT, 3 * 768], BF16, tag="wqkv", name="wqt")[:, :, :3 * D]
                nc.sync.dma_start(wq_sb, wqkv[l][:, :].rearrange("(ko p) d -> p ko d", p=128))
                qT = qk_pool.tile([128, KT, T], BF16, tag="qT")
                kTt = qk_pool.tile([128, KT, T], BF16, tag="kT")
                v_sb = v_pool.tile([128, NTB, E], BF16, tag="v")
                with tc.tile_pool(name=f"psqkv_{l}", bufs=4, space="PSUM") as pp:
                    for tc2 in range(2):
                        for dt in range(ndt):
                            dp = min(128, D - dt * 128)
                            for dst, off in ((qT, 0), (kTt, D)):
                                ps = pp.tile([128, 768], F32, tag="ps", name="pst")[:dp, :512]
                                for k in range(KT):
                                    nc.tensor.matmul(
                                        ps, lhsT=wq_sb[:, k, off + dt * 128: off + dt * 128 + dp],
                                        rhs=hT[:, k, tc2 * 512:(tc2 + 1) * 512],
                                        start=(k == 0), stop=(k == KT - 1))
                                nc.vector.tensor_copy(
                                    out=dst[:dp, dt, tc2 * 512:(tc2 + 1) * 512], in_=ps)
                    for st in range(NTB):
                        ps = pp.tile([128, 768], F32, tag="ps", name="pst")[:, :D]
                        for c0 in range(0, D, 512):
                            cn = min(512, D - c0)
                            for k in range(KT):
                                nc.tensor.matmul(
                                    ps[:, c0:c0 + cn],
                                    lhsT=hT[:, k, st * 128:(st + 1) * 128],
                                    rhs=wq_sb[:, k, 2 * D + c0:2 * D + c0 + cn],
                                    start=(k == 0), stop=(k == KT - 1))
                        nc.vector.tensor_copy(out=v_sb[:, st, :D], in_=ps)

                # ---- attention ----
                oT = oT_pool.tile([128, KT, T], BF16, tag="oT")
                with tc.tile_pool(name=f"psat_{l}", bufs=4, space="PSUM") as pp:
                    for hh in range(h):
                        dt, dr = hh // 2, 64 * (hh % 2)
                        v65 = v65_pool.tile([128, NTB, 65], BF16, tag="v65")
                        for st in range(NTB):
                            nc.gpsimd.tensor_copy(out=v65[:, st, :64],
                                                  in_=v_sb[:, st, hh * 64:(hh + 1) * 64])
                        nc.vector.memset(v65[:, :, 64:65], 1.0)
                        for tc2 in range(2):
                            smax = 4 * tc2 + 4
                            expT = ex_pool.tile([128, NTB, 512], BF16, tag="expT")
                            for st in range(smax):
                                ps = pp.tile([128, 768], F32, tag="ps", name="pst")[:, :512]
                                nc.tensor.matmul(
                                    ps, lhsT=kTt[dr:dr + 64, dt, st * 128:(st + 1) * 128],
                                    rhs=qT[dr:dr + 64, dt, tc2 * 512:(tc2 + 1) * 512],
                                    start=True, stop=True)
                                nc.scalar.activation(out=expT[:, st, :], in_=ps,
                                                     func=AF.Exp, scale=HS ** -0.5)
                                if st >= 4 * tc2:
                                    off = st * 128 - tc2 * 512
                                    if off > 0:
                                        nc.vector.memset(expT[:, st, :off], 0.0)
                                    nc.vector.tensor_tensor(
                                        out=expT[:, st, off:off + 128],
                                        in0=expT[:, st, off:off + 128],
                                        in1=cm_sb[:], op=OP.mult)
                            po = pp.tile([128, 768], F32, tag="ps", name="pst")[:65, :512]
                            for st in range(smax):
                                nc.tensor.matmul(po, lhsT=v65[:, st, :], rhs=expT[:, st, :],
                                                 start=(st == 0), stop=(st == smax - 1))
                            rc = rc_pool.tile([1, 512], F32, tag="rc")
                            nc.vector.reciprocal(rc[:], po[64:65, :])
                            pb = pp.tile([128, 768], F32, tag="ps", name="pst")[:64, :512]
                            nc.tensor.matmul(pb, lhsT=ones64[:], rhs=rc[:], start=True, stop=True)
                            rb = rc_pool.tile([64, 512], F32, tag="rb", name="rbt")
                            nc.scalar.copy(out=rb[:], in_=pb)
                            nc.vector.tensor_tensor(
                                out=oT[dr:dr + 64, dt, tc2 * 512:(tc2 + 1) * 512],
                                in0=po[:64, :], in1=rb[:], op=OP.mult)

                # ---- proj + residual ----
                pw_sb = pw_pool.tile([128, KT, E], BF16, tag="pw", name="pwt")[:, :ndt, :]
                nc.sync.dma_start(pw_sb, projw[l][:, :].rearrange("(ko p) e -> p ko e", p=128))
                with tc.tile_pool(name=f"pspr_{l}", bufs=2, space="PSUM") as pp:
                    for tb in range(NTB):
                        py = pp.tile([128, 768], F32, tag="ps", name="pst")
                        for k in range(ndt):
                            for c0, cn in ((0, 512), (512, 256)):
                                nc.tensor.matmul(py[:, c0:c0 + cn],
                                                 lhsT=oT[:, k, tb * 128:(tb + 1) * 128],
                                                 rhs=pw_sb[:, k, c0:c0 + cn],
                                                 start=(k == 0), stop=(k == ndt - 1))
                        nc.vector.tensor_tensor(out=x[:, tb, :], in0=x[:, tb, :],
                                                in1=gb_sb[:, 4, :], op=OP.add)
                        nc.vector.tensor_tensor(out=x[:, tb, :], in0=x[:, tb, :],
                                                in1=py[:], op=OP.add)

                # ---- ln2 ----
                hT = hT_pool.tile([128, KT, T], BF16, tag="hT")
                with tc.tile_pool(name=f"psln2_{l}", bufs=2, space="PSUM") as pp:
                    layer_norm(gb_sb, 2, 3, sc_pool, pp, hT)

                # ---- ffn (x += relu(hT.T@w1 + b1) @ w2 + b2) ----
                for tb in range(NTB):
                    nc.vector.tensor_tensor(out=x[:, tb, :], in0=x[:, tb, :],
                                            in1=gb_sb[:, 5, :], op=OP.add)
                nfc = (f + FC - 1) // FC
                with tc.tile_pool(name=f"psff_{l}", bufs=4, space="PSUM") as pp:
                    for fc in range(nfc):
                        fsz = min(FC, f - fc * FC)
                        nj = fsz // 128
                        w1c = w1_pool.tile([128, KT, FC], BF16, tag="w1c", name="w1t")[:, :, :fsz]
                        nc.sync.dma_start(
                            w1c, w1[l][:, fc * FC:fc * FC + fsz].rearrange("(ko p) f -> p ko f", p=128))
                        w2c = w2_pool.tile([128, FC // 128, E], BF16, tag="w2c", name="w2t")[:, :nj, :]
                        nc.sync.dma_start(
                            w2c, w2[l][fc * FC:fc * FC + fsz, :].rearrange("(ko p) e -> p ko e", p=128))
                        for th in range(2):
                            aT = aT_pool.tile([128, FC // 128, 512], BF16, tag="aT", name="aTt")[:, :nj, :]
                            for j in range(nj):
                                pa = pp.tile([128, 768], F32, tag="ps", name="pst")[:, :512]
                                for k in range(KT):
                                    nc.tensor.matmul(pa, lhsT=w1c[:, k, j * 128:(j + 1) * 128],
                                                     rhs=hT[:, k, th * 512:(th + 1) * 512],
                                                     start=(k == 0), stop=(k == KT - 1))
                                nc.scalar.activation(out=aT[:, j, :], in_=pa, func=AF.Relu,
                                                     bias=b1_sb[:, fc * 4 + j:fc * 4 + j + 1])
                            for tb in range(th * 4, th * 4 + 4):
                                py = pp.tile([128, 768], F32, tag="ps", name="pst")
                                for j in range(nj):
                                    for c0, cn in ((0, 512), (512, 256)):
                                        nc.tensor.matmul(
                                            py[:, c0:c0 + cn],
                                            lhsT=aT[:, j, (tb % 4) * 128:(tb % 4 + 1) * 128],
                                            rhs=w2c[:, j, c0:c0 + cn],
                                            start=(j == 0), stop=(j == nj - 1))
                                nc.vector.tensor_tensor(out=x[:, tb, :], in0=x[:, tb, :],
                                                        in1=py[:], op=OP.add)

            # ---- final layernorm -> xfT ----
            gbf_sb = gb_pool.tile([128, 6, E], BF16, tag="gb", name="gbf")[:, :2, :]
            nc.sync.dma_start(gbf_sb, _bcast(lnfgb[:, :]))
            xfT = hT_pool.tile([128, KT, T], BF16, tag="hT")
            with tc.tile_pool(name="pslnf", bufs=2, space="PSUM") as pp:
                layer_norm(gbf_sb, 0, 1, sc_pool, pp, xfT)

        # ================= phase 2: lm_head + exp-sums ===================
        with ExitStack() as p2:
            lw_pool = p2.enter_context(tc.tile_pool(name="lw", bufs=3))
            lt_pool = p2.enter_context(tc.tile_pool(name="lt", bufs=6))
            bs_pool = p2.enter_context(tc.tile_pool(name="bs", bufs=2))
            es_pool = p2.enter_context(tc.tile_pool(name="es", bufs=3))
            sm_pool = p2.enter_context(tc.tile_pool(name="sm", bufs=8))
            sums = [sm_pool.tile([128, NVC], F32, tag="sums", name=f"sums{i}") for i in range(NTB)]
            with tc.tile_pool(name="psl", bufs=8, space="PSUM") as pp:
                for vc in range(NVC):
                    lwc = lw_pool.tile([128, KT, 512], BF16, tag="lw")
                    nc.sync.dma_start(
                        lwc[:], lmw[:, vc * 512:(vc + 1) * 512].rearrange("(ko p) v -> p ko v", p=128))
                    bsc = bs_pool.tile([128, 512], F32, tag="bs")
                    nc.sync.dma_start(bsc[:], _bcast(lmb[:, vc * 512:(vc + 1) * 512]))
                    for tb in range(NTB):
                        pl = pp.tile([128, 512], F32, tag="pl", name="plt")
                        for k in range(KT):
                            nc.tensor.matmul(pl[:], lhsT=xfT[:, k, tb * 128:(tb + 1) * 128],
                                             rhs=lwc[:, k, :], start=(k == 0), stop=(k == KT - 1))
                        lt = lt_pool.tile([128, 512], F32, tag="lt")
                        nc.vector.tensor_tensor(out=lt[:], in0=pl[:], in1=bsc[:], op=OP.add)
                        nc.sync.dma_start(logits[tb * 128:(tb + 1) * 128, vc * 512:(vc + 1) * 512], lt[:])
                        es = es_pool.tile([128, 512], BF16, tag="es")
                        nc.scalar.activation(out=es[:], in_=lt[:], func=AF.Exp,
                                             accum_out=sums[tb][:, vc:vc + 1])
                for tb in range(NTB):
                    nc.vector.reduce_sum(out=sums_final[:, tb:tb + 1], in_=sums[tb][:], axis=AX.X)
            nc.sync.dma_start(sumexp[:, :].rearrange("a b -> b a"), sums_final[:])

    nc.finalize()
    return nc


_NC_CACHE = [None]


def _to_bf16(a):
    return np.asarray(a, dtype=np.float32).astype(ml_dtypes.bfloat16)


def kernel(idx, targets, params):
    idx = np.asarray(idx)
    targets = np.asarray(targets)
    p = {k: (v if isinstance(v, (list, dict)) else np.asarray(v)) for k, v in params.items()}

    # ---- shared (batch-independent) input prep ----
    tok = _to_bf16(p["tok"])
    pos = _to_bf16(np.asarray(p["pos"])[:T])
    cm = np.tril(np.ones((128, 128), np.float32)).T  # m[s,t] = 1 if s <= t
    cm = cm.astype(ml_dtypes.bfloat16)
    shared = {"tok": tok, "pos": pos, "cmask": cm}
    for l in range(NL):
        blk = p["blocks"][l]
        h, f = HEADS[l], FFN[l]
        D = h * HS
        nkp = (D + 127) // 128
        wq = np.asarray(blk["wq"]).transpose(1, 0, 2).reshape(E, D)
        wk = np.asarray(blk["wk"]).transpose(1, 0, 2).reshape(E, D)
        wv = np.asarray(blk["wv"]).transpose(1, 0, 2).reshape(E, D)
        shared[f"wqkv{l}"] = _to_bf16(np.concatenate([wq, wk, wv], axis=1))
        pw = np.zeros((nkp * 128, E), np.float32)
        pw[:D] = np.asarray(blk["proj_w"])
        shared[f"projw{l}"] = pw.astype(ml_dtypes.bfloat16)
        shared[f"w1_{l}"] = _to_bf16(blk["w1"])
        shared[f"w2_{l}"] = _to_bf16(blk["w2"])
        shared[f"b1_{l}"] = np.asarray(blk["b1"], np.float32).reshape(f // 128, 128).T.copy()
        shared[f"lngb{l}"] = _to_bf16(np.stack([
            np.asarray(blk["ln1_g"]), np.asarray(blk["ln1_b"]),
            np.asarray(blk["ln2_g"]), np.asarray(blk["ln2_b"]),
            np.asarray(blk["proj_b"]), np.asarray(blk["b2"])]))
    shared["lnfgb"] = _to_bf16(np.stack([np.asarray(p["lnf_g"]), np.asarray(p["lnf_b"])]))

    lm_w = np.asarray(p["lm_w"], np.float32)
    lm_b = np.asarray(p["lm_b"], np.float32)
    halves = []
    for vh in range(2):
        lo, hi = (0, VHALF0) if vh == 0 else (VHALF0, V)
        wpad = np.zeros((E, VH), np.float32)
        wpad[:, :hi - lo] = lm_w[:, lo:hi]
        bpad = np.full((1, VH), -1e30, np.float32)
        bpad[0, :hi - lo] = lm_b[lo:hi]
        halves.append((wpad.astype(ml_dtypes.bfloat16), bpad))

    in_maps = []
    for c in range(8):
        b, vh = c % 4, c // 4
        m = dict(shared)
        m["idx32"] = np.asarray(idx[b], np.int32).reshape(NTB, 128).T.copy()
        m["lmw"], m["lmb"] = halves[vh]
        in_maps.append(m)

    if _NC_CACHE[0] is None:
        _NC_CACHE[0] = build_nc()
    res = run_bass_kernel_spmd(_NC_CACHE[0], in_maps, core_ids=list(range(8))).results

    # ---- assemble full logits + loss on host ----
    n1 = V - VHALF0
    logits_full = np.empty((B, T, V), np.float32)
    lse = np.empty((B, T), np.float64)
    for b in range(B):
        logits_full[b, :, :VHALF0] = res[b]["logits"][:, :VHALF0]
        logits_full[b, :, VHALF0:] = res[b + 4]["logits"][:, :n1]
        s0 = res[b]["sumexp"].reshape(T).astype(np.float64)
        s1 = res[b + 4]["sumexp"].reshape(T).astype(np.float64)
        lse[b] = np.log(s0 + s1)
    tgt = np.take_along_axis(logits_full, targets[..., None].astype(np.int64), axis=-1)[..., 0]
    loss = np.float32((lse - tgt).mean())
    return logits_full, loss
